# revision 104
# baseline (speedup 1.0000x reference)
"""Trainium2 Bass kernel for nn_CoreRNNFW (fast-weight RNN with inner recall loop).

Strategy (v5): transposed layout, two staggered half-batch chains, and a
deferred-rsqrt trick -- relu((x-mu)*rs) == relu(x-mu)*rs for rs>0, and a
per-column scale commutes through the PE contraction, so inner steps consume
the unscaled relu output immediately while rs folds into the next step's
mask*coefficient tensor (computed off-chain on GPSIMD). Only the last inner
step of each timestep materializes true h for the weight matmuls/appends.
Other notes:
- Pure data parallel over batch B=256 -> 32 samples per core on 8 cores.
- All recurrent state lives in [feature-on-partitions, batch-on-free] layout:
    hT  [128, 4, 32]  = h[j, b], j = jc*128 + p
  so every matmul is a [128,<=128] lhsT x [128, 32] rhs -> tiny 32-row PE op,
  and no per-step transposes are needed.
- Hebbian fast-weights stay factored: A_t = sum_tau c_tau u_tau u_tau^T.
  U kept in two layouts (Ujb [j, q], Upi [q, i]); A@h is two PE contractions
  with a diagonal mask+coefficient multiply in between.
- LayerNorm stats via PE: lhsT = ones/512 -> E[x], E[x^2] broadcast across all
  128 partitions in PSUM; normalize is a short chain of [128, 4, 32] vector
  ops (full partition occupancy); rsqrt on the scalar engine.
- Everything bf16 into the PE; PSUM accumulation fp32.
- Kernel build is specialized on whether ln_gamma/ln_beta/b_h/head_b are
  trivial (they are for the reference initializer); the general path is
  compiled when they are not.
"""
import sys

sys.path.insert(0, "/opt/trn_rl_repo")

import numpy as np
import ml_dtypes
import concourse.bass as bass
import concourse.bacc as bacc
import concourse.tile as tile
from concourse import mybir
from concourse.bass_utils import run_bass_kernel_spmd

T, B, D_G, D_H, D_OUT = 24, 256, 256, 512, 256
S_INNER = 3
LAM, ETA = 0.95, 0.5
LN_EPS = 1e-5
N_CORES = 8
BC = B // N_CORES            # 32 samples per core
NQ = T * BC                  # 768 q-slots (tau-major: q = tau*32 + b)
NKC = NQ // 128              # 6 q-chunks of 128
F32 = mybir.dt.float32
BF16 = mybir.dt.bfloat16
AL = mybir.AluOpType
AF = mybir.ActivationFunctionType


def _bc_mid(ap2d, n):
    """[P, F] AP -> [P, n, F] with stride-0 broadcast over the middle dim."""
    return bass.AP(tensor=ap2d.tensor, offset=ap2d.offset,
                   ap=[ap2d.ap[0], [0, n], ap2d.ap[1]])


def _bc_last(ap2d, n):
    """[P, F] AP -> [P, F, n] with stride-0 broadcast over the last dim."""
    return bass.AP(tensor=ap2d.tensor, offset=ap2d.offset,
                   ap=[ap2d.ap[0], ap2d.ap[1], [0, n]])


def _build_nc(affine=False, has_bh=False, has_hb=False):
    nc = bacc.Bacc(None, target_bir_lowering=False, debug=False)

    zT = nc.dram_tensor("zT", [128, 2, T, BC], BF16, kind="ExternalInput")
    cleanv = nc.dram_tensor("cleanv", [BC, D_OUT], F32, kind="ExternalInput")
    WhT = nc.dram_tensor("WhT", [128, 4, D_H], BF16, kind="ExternalInput")
    WgT = nc.dram_tensor("WgT", [128, 2, D_H], BF16, kind="ExternalInput")
    HWT = nc.dram_tensor("HWT", [128, 4, D_OUT], BF16, kind="ExternalInput")
    id128 = nc.dram_tensor("id128", [128, 128], BF16, kind="ExternalInput")
    mask_r = nc.dram_tensor("mask_r", [128, NKC, BC], F32, kind="ExternalInput")
    gam_r = nc.dram_tensor("gam_r", [128, 4], F32, kind="ExternalInput")
    bet_r = nc.dram_tensor("bet_r", [128, 4], F32, kind="ExternalInput")
    bh_r = nc.dram_tensor("bh_r", [1, 4, 128], BF16, kind="ExternalInput")
    hbias_r = nc.dram_tensor("hbias_r", [BC, D_OUT], F32, kind="ExternalInput")

    partial = nc.dram_tensor("partial", [BC], F32, kind="ExternalOutput")

    with tile.TileContext(nc) as tc:
        with (
            tc.tile_pool(name="persist", bufs=1) as P,
            tc.tile_pool(name="work", bufs=3) as W,
            tc.tile_pool(name="stats", bufs=4) as ST,
            tc.tile_pool(name="psHB", bufs=1, space="PSUM") as PHB,
            tc.tile_pool(name="psX", bufs=1, space="PSUM") as PX,
            tc.tile_pool(name="psG", bufs=1, space="PSUM") as PG,
            tc.tile_pool(name="psST", bufs=1, space="PSUM") as PST,
            tc.tile_pool(name="psTR", bufs=1, space="PSUM") as PTR,
        ):
            # ---- persistent SBUF state ----
            z_sb = P.tile([128, 2, T, BC], BF16)      # z[g, t, b]
            WhT_sb = P.tile([128, 4, D_H], BF16)      # W_h^T[j, i]
            WgT_sb = P.tile([128, 2, D_H], BF16)      # W_g^T[g, i]
            HWT_sb = P.tile([128, 4, D_OUT], BF16)    # head_W^T[j, o]
            id_sb = P.tile([128, 128], BF16)
            ones_sb = P.tile([128, 128], BF16)        # 1/512 everywhere
            mask_sb = P.tile([128, NKC, BC], F32)

            Ujb = P.tile([128, 4, NQ], BF16)          # U[j, q]
            Upi = P.tile([128, NKC, D_H], BF16)       # U[q, i]
            c_sb = P.tile([128, NKC], F32)            # coefficients per q
            maskc = P.tile([128, NKC, BC], F32)       # mask * c (per timestep)

            hT = P.tile([128, 4, BC], BF16)           # current h, [j, b]
            hb_sb = P.tile([128, 4, BC], BF16)        # h_base, bf16 SBUF copy
            cv_sb = P.tile([BC, D_OUT], F32)
            hbias_sb = P.tile([BC, D_OUT], F32)
            tn_sb = P.tile([BC, D_OUT], F32)
            if affine:
                gam_sb = P.tile([128, 4], F32)
                bet_sb = P.tile([128, 4], F32)
            if has_bh:
                bh_sb = P.tile([1, 4, 128], BF16)     # lhsT chunks of b_h
                ones1_sb = P.tile([1, BC], BF16)

            # ---- input DMAs ----
            nc.sync.dma_start(out=z_sb, in_=zT[:])
            nc.sync.dma_start(out=WhT_sb, in_=WhT[:])
            nc.sync.dma_start(out=WgT_sb, in_=WgT[:])
            nc.sync.dma_start(out=HWT_sb, in_=HWT[:])
            nc.sync.dma_start(out=id_sb, in_=id128[:])
            nc.sync.dma_start(out=mask_sb, in_=mask_r[:])
            nc.sync.dma_start(out=cv_sb, in_=cleanv[:])
            if has_hb:
                nc.sync.dma_start(out=hbias_sb, in_=hbias_r[:])
            if affine:
                nc.sync.dma_start(out=gam_sb, in_=gam_r[:])
                nc.sync.dma_start(out=bet_sb, in_=bet_r[:])
            if has_bh:
                nc.sync.dma_start(out=bh_sb, in_=bh_r[:])
                nc.vector.memset(ones1_sb, 1.0)
            nc.vector.memset(ones_sb, 1.0 / D_H)
            nc.gpsimd.memset(Ujb, 0.0)
            nc.gpsimd.memset(Upi, 0.0)
            nc.vector.memset(c_sb, 0.0)
            nc.vector.memset(maskc, 0.0)

            HB = BC // 2   # half-batch of 16, two pipelined chains

            def ln_relu_T(x_ps, h, inner=False, defer=False):
                """LN+relu for half h. defer=True: return (zr, rs_) without
                materializing hT = zr*rs -- the rs scale is folded into the
                next step's masked-coefficient multiply (column scaling
                commutes through the PE contraction)."""
                bs = slice(h * HB, (h + 1) * HB)
                if inner:
                    # x_ps is the per-half [128, 4, HB] Ah accumulator
                    x0 = W.tile([128, 4, HB], BF16, tag=f"x0{h}")
                    nc.vector.tensor_add(x0, hb_sb[:, :, bs], x_ps)
                else:
                    x0 = hb_sb[:, :, bs]
                    nc.vector.tensor_copy(x0, x_ps[:, :, bs])
                xq = W.tile([128, 4, HB], BF16, tag=f"xq{h}")
                nc.gpsimd.tensor_mul(xq, x0, x0)
                st_ps = PST.tile([128, 2, HB], F32, tag=f"st{h}")
                for ic in range(4):
                    nc.tensor.matmul(st_ps[:, 0, :], ones_sb,
                                     x0[:, ic, :],
                                     start=(ic == 0), stop=(ic == 3))
                for ic in range(4):
                    nc.tensor.matmul(st_ps[:, 1, :], ones_sb,
                                     xq[:, ic, :],
                                     start=(ic == 0), stop=(ic == 3))
                st_sb = ST.tile([128, 2, HB], BF16, tag=f"stc{h}")
                nc.vector.tensor_copy(st_sb, st_ps)
                mu = st_sb[:, 0, :]
                sq = st_sb[:, 1, :]
                t_ = ST.tile([128, HB], BF16, tag=f"t{h}")
                v_ = ST.tile([128, HB], F32, tag=f"v{h}")
                vr_ = ST.tile([128, HB], F32, tag=f"vr{h}")
                rs_ = ST.tile([128, HB], BF16, tag=f"rs{h}")
                if defer:
                    # rs path is hidden behind the next step's matmuls
                    nc.gpsimd.tensor_mul(t_, mu, mu)
                    nc.gpsimd.tensor_sub(v_, sq, t_)
                else:
                    # rs gates the next timestep: keep the whole variance
                    # path on DVE right after the stats copy (no hops)
                    nc.vector.tensor_mul(t_, mu, mu)
                    nc.vector.tensor_sub(v_, sq, t_)
                nc.vector.reciprocal_approx_fast(vr_, v_)
                nc.scalar.activation(rs_, vr_, AF.Sqrt)
                z1 = W.tile([128, 4, HB], BF16, tag=f"z1{h}")
                nc.gpsimd.tensor_sub(z1, x0, _bc_mid(mu, 4))
                hTh = hT[:, :, bs]
                if affine:
                    z2 = W.tile([128, 4, HB], BF16, tag=f"z2{h}")
                    nc.vector.tensor_mul(z2, z1, _bc_mid(rs_[:, :], 4))
                    z3 = W.tile([128, 4, HB], BF16, tag=f"z3{h}")
                    nc.gpsimd.tensor_mul(z3, z2, _bc_last(gam_sb[:, :], HB))
                    z4 = W.tile([128, 4, HB], BF16, tag=f"z4{h}")
                    nc.vector.tensor_add(z4, z3, _bc_last(bet_sb[:, :], HB))
                    nc.vector.tensor_scalar_max(hTh, z4, 0.0)
                    return None, None
                zr = W.tile([128, 4, HB], BF16, tag=f"zr{h}")
                nc.gpsimd.tensor_scalar_max(zr, z1, 0.0)
                if defer:
                    return zr, rs_
                # rs > 0, so relu((x-mu)*rs) == relu(x-mu)*rs
                nc.gpsimd.tensor_mul(hTh, zr, _bc_mid(rs_[:, :], 4))
                return None, None

            # ---- normalized target (off critical path) ----
            def normalize(v_sb, out_sb):
                scr = W.tile([BC, D_OUT], F32, tag="nsq")
                ss = ST.tile([BC, 1], F32, tag="ss")
                nc.scalar.activation(scr, v_sb, AF.Square, accum_out=ss)
                nc.scalar.activation(ss, ss, AF.Sqrt)
                nc.vector.tensor_scalar_add(ss, ss, 1e-6)
                rr = ST.tile([BC, 1], F32, tag="rr")
                nc.vector.reciprocal(rr, ss)
                nc.vector.tensor_scalar_mul(out_sb, v_sb, rr)

            normalize(cv_sb, tn_sb)

            # ---- main time loop: two half-batch chains, chain 1 emitted
            # one stage behind chain 0 so the engines serve them offset ----
            def hb_mms(hb_ps, t, h):
                bs = slice(h * HB, (h + 1) * HB)
                for ic in range(4):
                    if has_bh:
                        nc.tensor.matmul(
                            hb_ps[:, ic, bs], bh_sb[:, ic, :],
                            ones1_sb[:, bs], start=True, stop=False)
                    for gc in range(2):
                        nc.tensor.matmul(
                            hb_ps[:, ic, bs],
                            WgT_sb[:, gc, ic * 128:(ic + 1) * 128],
                            z_sb[:, gc, t, bs],
                            start=(gc == 0 and not has_bh),
                            stop=(gc == 1 and t == 0))
                    if t > 0:
                        for jc in range(4):
                            nc.tensor.matmul(
                                hb_ps[:, ic, bs],
                                WhT_sb[:, jc, ic * 128:(ic + 1) * 128],
                                hT[:, jc, bs],
                                start=False, stop=(jc == 3))

            def append_full(psT, t):
                # u_{t-1}: hT still holds h_{t-1} for both halves here.
                q0 = BC * (t - 1)
                k0, p0 = q0 // 128, q0 % 128
                nc.gpsimd.tensor_copy(Ujb[:, :, q0:q0 + BC], hT)
                for jc in range(4):
                    nc.tensor.transpose(psT[0:BC, jc, :], hT[:, jc, :], id_sb)
                nc.scalar.copy(Upi[p0:p0 + BC, k0, :], psT[0:BC, :, :])
                nc.gpsimd.tensor_scalar_mul(c_sb, c_sb, LAM)
                nc.gpsimd.memset(c_sb[p0:p0 + BC, k0:k0 + 1], ETA)
                c3 = _bc_last(c_sb[:, :], BC)
                nc.gpsimd.tensor_mul(maskc, mask_sb, c3)

            def make_mrs(h, kt, rs_):
                """Fold the deferred rsqrt scale into the mask*coef tensor
                (off the critical path, on Pool)."""
                bs = slice(h * HB, (h + 1) * HB)
                mrs = W.tile([128, NKC, HB], BF16, tag=f"mrs{h}")
                nc.gpsimd.tensor_mul(mrs[:, :kt, :], maskc[:, :kt, bs],
                                     _bc_mid(rs_[:, :], kt))
                return mrs

            def inner_step(t, h, kt, src, mrs, defer):
                bs = slice(h * HB, (h + 1) * HB)
                G_ps = PG.tile([128, NKC, HB], F32, tag=f"G{h}")
                x_ps = PX.tile([128, 4, HB], F32, tag=f"x{h}")
                for k in range(kt):
                    for jc in range(4):
                        nc.tensor.matmul(
                            G_ps[:, k, :],
                            Ujb[:, jc, k * 128:(k + 1) * 128],
                            src[:, jc, :],
                            start=(jc == 0), stop=(jc == 3))
                e_ = W.tile([128, NKC, HB], BF16, tag=f"e{h}")
                nc.vector.tensor_mul(
                    e_[:, :kt, :], mrs[:, :kt, :], G_ps[:, :kt, :])
                for ic in range(4):
                    for k in range(kt):
                        nc.tensor.matmul(
                            x_ps[:, ic, :],
                            Upi[:, k, ic * 128:(ic + 1) * 128],
                            e_[:, k, :],
                            start=(k == 0), stop=(k == kt - 1))
                return ln_relu_T(x_ps, h, inner=True, defer=defer)

            for t in range(T):
                kt = (t * BC + 127) // 128
                hb_ps = PHB.tile([128, 4, BC], F32, tag="hb")
                hb_mms(hb_ps, t, 0)
                if t > 0:
                    psT = PTR.tile([64, 4, 128], BF16, tag="psT")
                    append_full(psT, t)
                dfo = (t > 0) and not affine   # defer the outer LN's scale
                st0 = ln_relu_T(hb_ps, 0, defer=dfo)       # h0 outer
                if t > 0 and not affine:
                    mrs0 = make_mrs(0, kt, st0[1])
                    st0 = inner_step(t, 0, kt, st0[0], mrs0, True)   # h0 i1
                elif t > 0:
                    inner_step(t, 0, kt, hT[:, :, 0:HB],
                               maskc[:, :, 0:HB], False)             # h0 i1
                hb_mms(hb_ps, t, 1)
                st1 = ln_relu_T(hb_ps, 1, defer=dfo)       # h1 outer
                if t > 0 and not affine:
                    mrs0 = make_mrs(0, kt, st0[1])
                    st0 = inner_step(t, 0, kt, st0[0], mrs0, True)   # h0 i2
                    mrs1 = make_mrs(1, kt, st1[1])
                    st1 = inner_step(t, 1, kt, st1[0], mrs1, True)   # h1 i1
                    mrs0 = make_mrs(0, kt, st0[1])
                    inner_step(t, 0, kt, st0[0], mrs0, False)        # h0 i3
                    mrs1 = make_mrs(1, kt, st1[1])
                    st1 = inner_step(t, 1, kt, st1[0], mrs1, True)   # h1 i2
                    mrs1 = make_mrs(1, kt, st1[1])
                    inner_step(t, 1, kt, st1[0], mrs1, False)        # h1 i3
                elif t > 0:
                    for _s, h in ((1, 1), (2, 0), (2, 1), (3, 0), (3, 1)):
                        bs0 = slice(h * HB, (h + 1) * HB)
                        inner_step(t, h, kt, hT[:, :, bs0],
                                   maskc[:, :, bs0], False)

            # ---- head + loss partials ----
            pp = PX.tile([128, 2, BC], F32, tag="x0")
            for oc in range(2):
                for jc in range(4):
                    nc.tensor.matmul(
                        pp[:, oc, :],
                        HWT_sb[:, jc, oc * 128:(oc + 1) * 128],
                        hT[:, jc, :],
                        start=(jc == 0), stop=(jc == 3))
            pb = W.tile([128, 2, BC], BF16, tag="pb")
            nc.vector.tensor_copy(pb, pp)
            psP = PTR.tile([64, 4, 128], BF16, tag="psT")
            for oc in range(2):
                nc.tensor.transpose(psP[0:BC, oc, :], pb[:, oc, :], id_sb)
            pred = W.tile([BC, D_OUT], F32, tag="pred")
            if has_hb:
                nc.vector.tensor_add(pred, hbias_sb, psP[0:BC, 0:2, :])
            else:
                nc.vector.tensor_copy(pred, psP[0:BC, 0:2, :])
            pn = W.tile([BC, D_OUT], F32, tag="pn")
            normalize(pred, pn)
            diff = W.tile([BC, D_OUT], F32, tag="diff")
            nc.vector.tensor_sub(diff, pn, tn_sb)
            dsq = W.tile([BC, D_OUT], F32, tag="dsq")
            dss = ST.tile([BC, 1], F32, tag="dss")
            nc.scalar.activation(dsq, diff, AF.Square, accum_out=dss)
            nc.sync.dma_start(out=partial[:], in_=dss[:, 0])

    nc.compile()
    return nc


_NC_CACHE = {}


def _get_nc(affine=False, has_bh=False, has_hb=False):
    key = (affine, has_bh, has_hb)
    if key not in _NC_CACHE:
        _NC_CACHE[key] = _build_nc(*key)
    return _NC_CACHE[key]


def _flags(b_h, ln_gamma, ln_beta, head_b):
    affine = not (np.all(ln_gamma == 1.0) and np.all(ln_beta == 0.0))
    has_bh = not np.all(b_h == 0.0)
    has_hb = not np.all(head_b == 0.0)
    return bool(affine), bool(has_bh), bool(has_hb)


def _make_in_maps(inputs):
    return _prep_in_maps(**inputs)[1]


def _prep_in_maps(z_seq, clean_vec, W_h, W_g, b_h, ln_gamma, ln_beta, head_W,
                  head_b):
    z_seq = np.ascontiguousarray(np.asarray(z_seq, np.float32))
    clean_vec = np.ascontiguousarray(np.asarray(clean_vec, np.float32))
    W_h = np.asarray(W_h, np.float32)
    W_g = np.asarray(W_g, np.float32)
    b_h = np.asarray(b_h, np.float32)
    ln_gamma = np.asarray(ln_gamma, np.float32)
    ln_beta = np.asarray(ln_beta, np.float32)
    head_W = np.asarray(head_W, np.float32)
    head_b = np.asarray(head_b, np.float32)

    flags = _flags(b_h, ln_gamma, ln_beta, head_b)

    def chunk_w(wt, nck):  # [J, I] -> [128, nck, I], J = ck*128 + p
        J, I = wt.shape
        return np.ascontiguousarray(
            wt.reshape(nck, 128, I).transpose(1, 0, 2)).astype(
                ml_dtypes.bfloat16)

    WhT = chunk_w(W_h.T, 4)
    WgT = chunk_w(W_g.T, 2)
    HWT = chunk_w(head_W.T, 4)
    id128 = np.eye(128, dtype=ml_dtypes.bfloat16)
    mask = (np.arange(128)[:, None] % BC == np.arange(BC)[None, :])
    mask_r = np.ascontiguousarray(
        np.broadcast_to(mask[:, None, :], (128, NKC, BC)).astype(np.float32))
    gam_r = np.ascontiguousarray(ln_gamma.reshape(4, 128).T)
    bet_r = np.ascontiguousarray(ln_beta.reshape(4, 128).T)
    bh_r = np.ascontiguousarray(b_h.reshape(1, 4, 128)).astype(
        ml_dtypes.bfloat16)
    hbias_r = np.ascontiguousarray(np.tile(head_b[None, :], (BC, 1)))

    in_maps = []
    for m in range(N_CORES):
        sl = slice(m * BC, (m + 1) * BC)
        in_maps.append({
            "zT": np.ascontiguousarray(
                z_seq[:, sl, :].transpose(2, 0, 1).reshape(2, 128, T, BC)
                .transpose(1, 0, 2, 3)).astype(ml_dtypes.bfloat16),
            "cleanv": np.ascontiguousarray(clean_vec[sl]),
            "WhT": WhT, "WgT": WgT, "HWT": HWT,
            "id128": id128, "mask_r": mask_r,
            "gam_r": gam_r, "bet_r": bet_r, "bh_r": bh_r,
            "hbias_r": hbias_r,
        })

    return flags, in_maps


def kernel(**inputs):
    flags, in_maps = _prep_in_maps(**inputs)
    nc = _get_nc(*flags)
    res = run_bass_kernel_spmd(nc, in_maps, list(range(N_CORES)))
    total = np.float64(0.0)
    for m in range(N_CORES):
        total += np.float64(res.results[m]["partial"].sum())
    loss = total / (B * D_OUT)
    return np.array(loss, dtype=np.float32)


if __name__ == "__main__":
    import reference as ref
    inputs = {k: np.asarray(v) for k, v in ref.setup_inputs().items()}
    out = kernel(**inputs)
    print("kernel loss:", out)


# revision 105
# speedup vs baseline: 1.0255x; 1.0255x over previous
"""Trainium2 Bass kernel for nn_CoreRNNFW (fast-weight RNN with inner recall loop).

Strategy (v5): transposed layout, two staggered half-batch chains, and a
deferred-rsqrt trick -- relu((x-mu)*rs) == relu(x-mu)*rs for rs>0, and a
per-column scale commutes through the PE contraction, so inner steps consume
the unscaled relu output immediately while rs folds into the next step's
mask*coefficient tensor (computed off-chain on GPSIMD). Only the last inner
step of each timestep materializes true h for the weight matmuls/appends.
Other notes:
- Pure data parallel over batch B=256 -> 32 samples per core on 8 cores.
- All recurrent state lives in [feature-on-partitions, batch-on-free] layout:
    hT  [128, 4, 32]  = h[j, b], j = jc*128 + p
  so every matmul is a [128,<=128] lhsT x [128, 32] rhs -> tiny 32-row PE op,
  and no per-step transposes are needed.
- Hebbian fast-weights stay factored: A_t = sum_tau c_tau u_tau u_tau^T.
  U kept in two layouts (Ujb [j, q], Upi [q, i]); A@h is two PE contractions
  with a diagonal mask+coefficient multiply in between.
- LayerNorm stats via PE: lhsT = ones/512 -> E[x], E[x^2] broadcast across all
  128 partitions in PSUM; normalize is a short chain of [128, 4, 32] vector
  ops (full partition occupancy); rsqrt on the scalar engine.
- Everything bf16 into the PE; PSUM accumulation fp32.
- Kernel build is specialized on whether ln_gamma/ln_beta/b_h/head_b are
  trivial (they are for the reference initializer); the general path is
  compiled when they are not.
"""
import sys

sys.path.insert(0, "/opt/trn_rl_repo")

import numpy as np
import ml_dtypes
import concourse.bass as bass
import concourse.bacc as bacc
import concourse.tile as tile
from concourse import mybir
from concourse.bass_utils import run_bass_kernel_spmd

T, B, D_G, D_H, D_OUT = 24, 256, 256, 512, 256
S_INNER = 3
LAM, ETA = 0.95, 0.5
LN_EPS = 1e-5
N_CORES = 8
BC = B // N_CORES            # 32 samples per core
NQ = T * BC                  # 768 q-slots (tau-major: q = tau*32 + b)
NKC = NQ // 128              # 6 q-chunks of 128
F32 = mybir.dt.float32
BF16 = mybir.dt.bfloat16
AL = mybir.AluOpType
AF = mybir.ActivationFunctionType


def _bc_mid(ap2d, n):
    """[P, F] AP -> [P, n, F] with stride-0 broadcast over the middle dim."""
    return bass.AP(tensor=ap2d.tensor, offset=ap2d.offset,
                   ap=[ap2d.ap[0], [0, n], ap2d.ap[1]])


def _bc_last(ap2d, n):
    """[P, F] AP -> [P, F, n] with stride-0 broadcast over the last dim."""
    return bass.AP(tensor=ap2d.tensor, offset=ap2d.offset,
                   ap=[ap2d.ap[0], ap2d.ap[1], [0, n]])


def _build_nc(affine=False, has_bh=False, has_hb=False):
    nc = bacc.Bacc(None, target_bir_lowering=False, debug=False)

    zT = nc.dram_tensor("zT", [128, 2, T, BC], BF16, kind="ExternalInput")
    cleanv = nc.dram_tensor("cleanv", [BC, D_OUT], F32, kind="ExternalInput")
    WhT = nc.dram_tensor("WhT", [128, 4, D_H], BF16, kind="ExternalInput")
    WgT = nc.dram_tensor("WgT", [128, 2, D_H], BF16, kind="ExternalInput")
    HWT = nc.dram_tensor("HWT", [128, 4, D_OUT], BF16, kind="ExternalInput")
    id128 = nc.dram_tensor("id128", [128, 128], BF16, kind="ExternalInput")
    mask_r = nc.dram_tensor("mask_r", [128, NKC, BC], F32, kind="ExternalInput")
    gam_r = nc.dram_tensor("gam_r", [128, 4], F32, kind="ExternalInput")
    bet_r = nc.dram_tensor("bet_r", [128, 4], F32, kind="ExternalInput")
    bh_r = nc.dram_tensor("bh_r", [1, 4, 128], BF16, kind="ExternalInput")
    hbias_r = nc.dram_tensor("hbias_r", [BC, D_OUT], F32, kind="ExternalInput")

    partial = nc.dram_tensor("partial", [BC], F32, kind="ExternalOutput")

    with tile.TileContext(nc) as tc:
        with (
            tc.tile_pool(name="persist", bufs=1) as P,
            tc.tile_pool(name="work", bufs=3) as W,
            tc.tile_pool(name="stats", bufs=4) as ST,
            tc.tile_pool(name="psHB", bufs=1, space="PSUM") as PHB,
            tc.tile_pool(name="psX", bufs=1, space="PSUM") as PX,
            tc.tile_pool(name="psG", bufs=1, space="PSUM") as PG,
            tc.tile_pool(name="psST", bufs=1, space="PSUM") as PST,
            tc.tile_pool(name="psTR", bufs=1, space="PSUM") as PTR,
        ):
            # ---- persistent SBUF state ----
            z_sb = P.tile([128, 2, T, BC], BF16)      # z[g, t, b]
            WhT_sb = P.tile([128, 4, D_H], BF16)      # W_h^T[j, i]
            WgT_sb = P.tile([128, 2, D_H], BF16)      # W_g^T[g, i]
            HWT_sb = P.tile([128, 4, D_OUT], BF16)    # head_W^T[j, o]
            id_sb = P.tile([128, 128], BF16)
            ones_sb = P.tile([128, 128], BF16)        # 1/512 everywhere
            mask_sb = P.tile([128, NKC, BC], F32)

            Ujb = P.tile([128, 4, NQ], BF16)          # U[j, q]
            Upi = P.tile([128, NKC, D_H], BF16)       # U[q, i]
            c_sb = P.tile([128, NKC], F32)            # coefficients per q
            maskc = P.tile([128, NKC, BC], F32)       # mask * c (per timestep)

            hT = P.tile([128, 4, BC], BF16)           # current h, [j, b]
            hb_sb = P.tile([128, 4, BC], BF16)        # h_base, bf16 SBUF copy
            cv_sb = P.tile([BC, D_OUT], F32)
            hbias_sb = P.tile([BC, D_OUT], F32)
            tn_sb = P.tile([BC, D_OUT], F32)
            if affine:
                gam_sb = P.tile([128, 4], F32)
                bet_sb = P.tile([128, 4], F32)
            if has_bh:
                bh_sb = P.tile([1, 4, 128], BF16)     # lhsT chunks of b_h
                ones1_sb = P.tile([1, BC], BF16)

            # ---- input DMAs ----
            nc.sync.dma_start(out=z_sb, in_=zT[:])
            nc.sync.dma_start(out=WhT_sb, in_=WhT[:])
            nc.sync.dma_start(out=WgT_sb, in_=WgT[:])
            nc.sync.dma_start(out=HWT_sb, in_=HWT[:])
            nc.sync.dma_start(out=id_sb, in_=id128[:])
            nc.sync.dma_start(out=mask_sb, in_=mask_r[:])
            nc.sync.dma_start(out=cv_sb, in_=cleanv[:])
            if has_hb:
                nc.sync.dma_start(out=hbias_sb, in_=hbias_r[:])
            if affine:
                nc.sync.dma_start(out=gam_sb, in_=gam_r[:])
                nc.sync.dma_start(out=bet_sb, in_=bet_r[:])
            if has_bh:
                nc.sync.dma_start(out=bh_sb, in_=bh_r[:])
                nc.vector.memset(ones1_sb, 1.0)
            nc.vector.memset(ones_sb, 1.0 / D_H)
            nc.gpsimd.memset(Ujb, 0.0)
            nc.gpsimd.memset(Upi, 0.0)
            nc.vector.memset(c_sb, 0.0)
            nc.vector.memset(maskc, 0.0)

            HB = BC // 2   # half-batch of 16, two pipelined chains

            def ln_relu_T(x_ps, h, inner=False, defer=False):
                """LN+relu for half h. defer=True: return (zr, rs_) without
                materializing hT = zr*rs -- the rs scale is folded into the
                next step's masked-coefficient multiply (column scaling
                commutes through the PE contraction)."""
                bs = slice(h * HB, (h + 1) * HB)
                if inner:
                    # x_ps is the per-half [128, 4, HB] Ah accumulator
                    x0 = W.tile([128, 4, HB], BF16, tag=f"x0{h}")
                    nc.vector.tensor_add(x0, hb_sb[:, :, bs], x_ps)
                else:
                    x0 = hb_sb[:, :, bs]
                    nc.vector.tensor_copy(x0, x_ps[:, :, bs])
                xq = W.tile([128, 4, HB], BF16, tag=f"xq{h}")
                nc.gpsimd.tensor_mul(xq, x0, x0)
                st_ps = PST.tile([128, 2, HB], F32, tag=f"st{h}")
                for ic in range(4):
                    nc.tensor.matmul(st_ps[:, 0, :], ones_sb,
                                     x0[:, ic, :],
                                     start=(ic == 0), stop=(ic == 3))
                for ic in range(4):
                    nc.tensor.matmul(st_ps[:, 1, :], ones_sb,
                                     xq[:, ic, :],
                                     start=(ic == 0), stop=(ic == 3))
                st_sb = ST.tile([128, 2, HB], BF16, tag=f"stc{h}")
                nc.vector.tensor_copy(st_sb, st_ps)
                mu = st_sb[:, 0, :]
                sq = st_sb[:, 1, :]
                t_ = ST.tile([128, HB], BF16, tag=f"t{h}")
                v_ = ST.tile([128, HB], F32, tag=f"v{h}")
                vr_ = ST.tile([128, HB], F32, tag=f"vr{h}")
                rs_ = ST.tile([128, HB], BF16, tag=f"rs{h}")
                if defer:
                    # rs path is hidden behind the next step's matmuls
                    nc.gpsimd.tensor_mul(t_, mu, mu)
                    nc.gpsimd.tensor_sub(v_, sq, t_)
                else:
                    # rs gates the next timestep: keep the whole variance
                    # path on DVE right after the stats copy (no hops)
                    nc.vector.tensor_mul(t_, mu, mu)
                    nc.vector.tensor_sub(v_, sq, t_)
                nc.vector.reciprocal_approx_fast(vr_, v_)
                nc.scalar.activation(rs_, vr_, AF.Sqrt)
                z1 = W.tile([128, 4, HB], BF16, tag=f"z1{h}")
                nc.gpsimd.tensor_sub(z1, x0, _bc_mid(mu, 4))
                hTh = hT[:, :, bs]
                if affine:
                    z2 = W.tile([128, 4, HB], BF16, tag=f"z2{h}")
                    nc.vector.tensor_mul(z2, z1, _bc_mid(rs_[:, :], 4))
                    z3 = W.tile([128, 4, HB], BF16, tag=f"z3{h}")
                    nc.gpsimd.tensor_mul(z3, z2, _bc_last(gam_sb[:, :], HB))
                    z4 = W.tile([128, 4, HB], BF16, tag=f"z4{h}")
                    nc.vector.tensor_add(z4, z3, _bc_last(bet_sb[:, :], HB))
                    nc.vector.tensor_scalar_max(hTh, z4, 0.0)
                    return None, None
                zr = W.tile([128, 4, HB], BF16, tag=f"zr{h}")
                nc.gpsimd.tensor_scalar_max(zr, z1, 0.0)
                if defer:
                    return zr, rs_
                # rs > 0, so relu((x-mu)*rs) == relu(x-mu)*rs
                nc.gpsimd.tensor_mul(hTh, zr, _bc_mid(rs_[:, :], 4))
                return None, None

            # ---- normalized target (off critical path) ----
            def normalize(v_sb, out_sb):
                scr = W.tile([BC, D_OUT], F32, tag="nsq")
                ss = ST.tile([BC, 1], F32, tag="ss")
                nc.scalar.activation(scr, v_sb, AF.Square, accum_out=ss)
                nc.scalar.activation(ss, ss, AF.Sqrt)
                nc.vector.tensor_scalar_add(ss, ss, 1e-6)
                rr = ST.tile([BC, 1], F32, tag="rr")
                nc.vector.reciprocal(rr, ss)
                nc.vector.tensor_scalar_mul(out_sb, v_sb, rr)

            normalize(cv_sb, tn_sb)

            # ---- main time loop: two half-batch chains, chain 1 emitted
            # one stage behind chain 0 so the engines serve them offset ----
            def hb_mms(hb_ps, t, h):
                bs = slice(h * HB, (h + 1) * HB)
                for ic in range(4):
                    if has_bh:
                        nc.tensor.matmul(
                            hb_ps[:, ic, bs], bh_sb[:, ic, :],
                            ones1_sb[:, bs], start=True, stop=False)
                    for gc in range(2):
                        nc.tensor.matmul(
                            hb_ps[:, ic, bs],
                            WgT_sb[:, gc, ic * 128:(ic + 1) * 128],
                            z_sb[:, gc, t, bs],
                            start=(gc == 0 and not has_bh),
                            stop=(gc == 1 and t == 0))
                    if t > 0:
                        for jc in range(4):
                            nc.tensor.matmul(
                                hb_ps[:, ic, bs],
                                WhT_sb[:, jc, ic * 128:(ic + 1) * 128],
                                hT[:, jc, bs],
                                start=False, stop=(jc == 3))

            def append_full(psT, t):
                # u_{t-1}: hT still holds h_{t-1} for both halves here.
                q0 = BC * (t - 1)
                k0, p0 = q0 // 128, q0 % 128
                nc.gpsimd.tensor_copy(Ujb[:, :, q0:q0 + BC], hT)
                for jc in range(4):
                    nc.tensor.transpose(psT[0:BC, jc, :], hT[:, jc, :], id_sb)
                nc.scalar.copy(Upi[p0:p0 + BC, k0, :], psT[0:BC, :, :])
                nc.gpsimd.tensor_scalar_mul(c_sb, c_sb, LAM)
                nc.gpsimd.memset(c_sb[p0:p0 + BC, k0:k0 + 1], ETA)
                c3 = _bc_last(c_sb[:, :], BC)
                nc.gpsimd.tensor_mul(maskc, mask_sb, c3)

            def make_mrs(h, kt, rs_):
                """Fold the deferred rsqrt scale into the mask*coef tensor
                (off the critical path, on Pool)."""
                bs = slice(h * HB, (h + 1) * HB)
                mrs = W.tile([128, NKC, HB], BF16, tag=f"mrs{h}")
                nc.vector.tensor_mul(mrs[:, :kt, :], maskc[:, :kt, bs],
                                     _bc_mid(rs_[:, :], kt))
                return mrs

            def inner_step(t, h, kt, src, mrs, defer):
                bs = slice(h * HB, (h + 1) * HB)
                G_ps = PG.tile([128, NKC, HB], F32, tag=f"G{h}")
                x_ps = PX.tile([128, 4, HB], F32, tag=f"x{h}")
                for k in range(kt):
                    for jc in range(4):
                        nc.tensor.matmul(
                            G_ps[:, k, :],
                            Ujb[:, jc, k * 128:(k + 1) * 128],
                            src[:, jc, :],
                            start=(jc == 0), stop=(jc == 3))
                e_ = W.tile([128, NKC, HB], BF16, tag=f"e{h}")
                nc.vector.tensor_mul(
                    e_[:, :kt, :], mrs[:, :kt, :], G_ps[:, :kt, :])
                for ic in range(4):
                    for k in range(kt):
                        nc.tensor.matmul(
                            x_ps[:, ic, :],
                            Upi[:, k, ic * 128:(ic + 1) * 128],
                            e_[:, k, :],
                            start=(k == 0), stop=(k == kt - 1))
                return ln_relu_T(x_ps, h, inner=True, defer=defer)

            for t in range(T):
                kt = (t * BC + 127) // 128
                hb_ps = PHB.tile([128, 4, BC], F32, tag="hb")
                hb_mms(hb_ps, t, 0)
                if t > 0:
                    psT = PTR.tile([64, 4, 128], BF16, tag="psT")
                    append_full(psT, t)
                dfo = (t > 0) and not affine   # defer the outer LN's scale
                st0 = ln_relu_T(hb_ps, 0, defer=dfo)       # h0 outer
                if t > 0 and not affine:
                    mrs0 = make_mrs(0, kt, st0[1])
                    st0 = inner_step(t, 0, kt, st0[0], mrs0, True)   # h0 i1
                elif t > 0:
                    inner_step(t, 0, kt, hT[:, :, 0:HB],
                               maskc[:, :, 0:HB], False)             # h0 i1
                hb_mms(hb_ps, t, 1)
                st1 = ln_relu_T(hb_ps, 1, defer=dfo)       # h1 outer
                if t > 0 and not affine:
                    mrs0 = make_mrs(0, kt, st0[1])
                    st0 = inner_step(t, 0, kt, st0[0], mrs0, True)   # h0 i2
                    mrs1 = make_mrs(1, kt, st1[1])
                    st1 = inner_step(t, 1, kt, st1[0], mrs1, True)   # h1 i1
                    mrs0 = make_mrs(0, kt, st0[1])
                    inner_step(t, 0, kt, st0[0], mrs0, False)        # h0 i3
                    mrs1 = make_mrs(1, kt, st1[1])
                    st1 = inner_step(t, 1, kt, st1[0], mrs1, True)   # h1 i2
                    mrs1 = make_mrs(1, kt, st1[1])
                    inner_step(t, 1, kt, st1[0], mrs1, False)        # h1 i3
                elif t > 0:
                    for _s, h in ((1, 1), (2, 0), (2, 1), (3, 0), (3, 1)):
                        bs0 = slice(h * HB, (h + 1) * HB)
                        inner_step(t, h, kt, hT[:, :, bs0],
                                   maskc[:, :, bs0], False)

            # ---- head + loss partials ----
            pp = PX.tile([128, 2, BC], F32, tag="x0")
            for oc in range(2):
                for jc in range(4):
                    nc.tensor.matmul(
                        pp[:, oc, :],
                        HWT_sb[:, jc, oc * 128:(oc + 1) * 128],
                        hT[:, jc, :],
                        start=(jc == 0), stop=(jc == 3))
            pb = W.tile([128, 2, BC], BF16, tag="pb")
            nc.vector.tensor_copy(pb, pp)
            psP = PTR.tile([64, 4, 128], BF16, tag="psT")
            for oc in range(2):
                nc.tensor.transpose(psP[0:BC, oc, :], pb[:, oc, :], id_sb)
            pred = W.tile([BC, D_OUT], F32, tag="pred")
            if has_hb:
                nc.vector.tensor_add(pred, hbias_sb, psP[0:BC, 0:2, :])
            else:
                nc.vector.tensor_copy(pred, psP[0:BC, 0:2, :])
            pn = W.tile([BC, D_OUT], F32, tag="pn")
            normalize(pred, pn)
            diff = W.tile([BC, D_OUT], F32, tag="diff")
            nc.vector.tensor_sub(diff, pn, tn_sb)
            dsq = W.tile([BC, D_OUT], F32, tag="dsq")
            dss = ST.tile([BC, 1], F32, tag="dss")
            nc.scalar.activation(dsq, diff, AF.Square, accum_out=dss)
            nc.sync.dma_start(out=partial[:], in_=dss[:, 0])

    nc.compile()
    return nc


_NC_CACHE = {}


def _get_nc(affine=False, has_bh=False, has_hb=False):
    key = (affine, has_bh, has_hb)
    if key not in _NC_CACHE:
        _NC_CACHE[key] = _build_nc(*key)
    return _NC_CACHE[key]


def _flags(b_h, ln_gamma, ln_beta, head_b):
    affine = not (np.all(ln_gamma == 1.0) and np.all(ln_beta == 0.0))
    has_bh = not np.all(b_h == 0.0)
    has_hb = not np.all(head_b == 0.0)
    return bool(affine), bool(has_bh), bool(has_hb)


def _make_in_maps(inputs):
    return _prep_in_maps(**inputs)[1]


def _prep_in_maps(z_seq, clean_vec, W_h, W_g, b_h, ln_gamma, ln_beta, head_W,
                  head_b):
    z_seq = np.ascontiguousarray(np.asarray(z_seq, np.float32))
    clean_vec = np.ascontiguousarray(np.asarray(clean_vec, np.float32))
    W_h = np.asarray(W_h, np.float32)
    W_g = np.asarray(W_g, np.float32)
    b_h = np.asarray(b_h, np.float32)
    ln_gamma = np.asarray(ln_gamma, np.float32)
    ln_beta = np.asarray(ln_beta, np.float32)
    head_W = np.asarray(head_W, np.float32)
    head_b = np.asarray(head_b, np.float32)

    flags = _flags(b_h, ln_gamma, ln_beta, head_b)

    def chunk_w(wt, nck):  # [J, I] -> [128, nck, I], J = ck*128 + p
        J, I = wt.shape
        return np.ascontiguousarray(
            wt.reshape(nck, 128, I).transpose(1, 0, 2)).astype(
                ml_dtypes.bfloat16)

    WhT = chunk_w(W_h.T, 4)
    WgT = chunk_w(W_g.T, 2)
    HWT = chunk_w(head_W.T, 4)
    id128 = np.eye(128, dtype=ml_dtypes.bfloat16)
    mask = (np.arange(128)[:, None] % BC == np.arange(BC)[None, :])
    mask_r = np.ascontiguousarray(
        np.broadcast_to(mask[:, None, :], (128, NKC, BC)).astype(np.float32))
    gam_r = np.ascontiguousarray(ln_gamma.reshape(4, 128).T)
    bet_r = np.ascontiguousarray(ln_beta.reshape(4, 128).T)
    bh_r = np.ascontiguousarray(b_h.reshape(1, 4, 128)).astype(
        ml_dtypes.bfloat16)
    hbias_r = np.ascontiguousarray(np.tile(head_b[None, :], (BC, 1)))

    in_maps = []
    for m in range(N_CORES):
        sl = slice(m * BC, (m + 1) * BC)
        in_maps.append({
            "zT": np.ascontiguousarray(
                z_seq[:, sl, :].transpose(2, 0, 1).reshape(2, 128, T, BC)
                .transpose(1, 0, 2, 3)).astype(ml_dtypes.bfloat16),
            "cleanv": np.ascontiguousarray(clean_vec[sl]),
            "WhT": WhT, "WgT": WgT, "HWT": HWT,
            "id128": id128, "mask_r": mask_r,
            "gam_r": gam_r, "bet_r": bet_r, "bh_r": bh_r,
            "hbias_r": hbias_r,
        })

    return flags, in_maps


def kernel(**inputs):
    flags, in_maps = _prep_in_maps(**inputs)
    nc = _get_nc(*flags)
    res = run_bass_kernel_spmd(nc, in_maps, list(range(N_CORES)))
    total = np.float64(0.0)
    for m in range(N_CORES):
        total += np.float64(res.results[m]["partial"].sum())
    loss = total / (B * D_OUT)
    return np.array(loss, dtype=np.float32)


if __name__ == "__main__":
    import reference as ref
    inputs = {k: np.asarray(v) for k, v in ref.setup_inputs().items()}
    out = kernel(**inputs)
    print("kernel loss:", out)
